# revision 60
# baseline (speedup 1.0000x reference)
#!/usr/bin/env python3
"""Bass/Trainium2 kernel for nn_Attention_12747462934680.

Reference computation (B=64, L=2048, H=512):
    x = concat([hidden broadcast over L, encoder_outputs], -1)   # [B, L, 2H]
    energy = tanh(x @ W.T + b)                                   # [B, L, H]
    scores = energy @ v                                          # [B, L]
    attn = softmax(scores, axis=1)[:, None, :]                   # [B, 1, L]

Decomposition:
    pre[b,l,h] = (enc[b,l] @ W2.T)[h] + (hidden[b] @ W1.T)[h] + bias[h]
    with W1 = W[:, :H], W2 = W[:, H:].  The hidden term is per-(b,h), computed
    once; the big matmul is enc @ W2.T.

Sharding: data-parallel over B across 8 cores (8 batches/core).

Per-core device pipeline (SPMD, no collectives).  All heavy data rides in
fp16 (~11-bit mantissa); enc is pre-transposed AND pre-cast on the host so
the kernel DMAs k-major [128, kt, t] tiles directly — no on-device
transposes at all.  Per group of 512 tokens (batch b, l-chunk j, j-outer):
  - preT[h, t] = W2T.T @ encT chunk (fp16 matmul, fp32 PSUM), 4x4 k/h tiles
  - energy = tanh(preT + h1T[:, b]) on ACT (PSUM -> SBUF, fp16), per h-tile
  - v-dot is split so the PE does 1/4 of the naive work:
      z[p, t] = sum_ht v[ht*128+p] * energy[ht][p, t]   (DVE-only tree:
      per-partition tensor_scalar muls run in 4x mode, adds in 2x)
      scores row b = onehot_b.T @ z  (single 512-row matmul; all 8 b of a
      chunk accumulate into one [8, 512] PSUM tile, one row per batch)
  - per chunk: full-rescale online softmax reads that PSUM directly:
    running max m, exp-chunk into fp16 expstore[8, L] (ACT, accum_out
    gives the chunk sum), stored chunks and s rescaled by exp(m_old-m_new)
  - the last chunk skips its reduce_max entirely (softmax is shift
    invariant, exp against the running max), and the last group's v-dots
    are direct matmuls with v-valued one-hot stationaries so no DVE chain
    separates its tanhs from the final PSUM
  - tail: exp -> s -> 1/s -> one fp16 4x multiply -> one [8, L] fp16 DMA
Latency details: a dependency-free PE warm-up chain pins the p-state ramp
clock before mm(0) is costed; w2t+enc0 lead the DMA queue so the PE stream
starts ~5us in; h1's params ride the same sync queue right after enc0 (the
tile scheduler is greedy ready-first, so h1 must not become ready first).
"""
import sys
import numpy as np

sys.path.insert(0, "/opt/trn_rl_repo")

B, L, H = 64, 2048, 512
NCORES = 8
BPC = B // NCORES          # batches per core
T = BPC * L                # tokens per core = 16384
GT = 512                   # tokens per group
G = T // GT                # 32 groups
NJ = L // GT               # 4 l-chunks per batch
KT = H // 128              # 4 k-tiles
HT = H // 128              # 4 h-tiles

_compiled = None


def _build(variant="full"):
    from contextlib import ExitStack
    from concourse import bacc, mybir
    import concourse.tile as tile
    from concourse.bass import ts

    f32 = mybir.dt.float32
    fp16 = mybir.dt.float16
    DT = fp16
    ActF = mybir.ActivationFunctionType

    nc = bacc.Bacc("TRN2", target_bir_lowering=False, debug=False,
                   enable_asserts=True, num_devices=NCORES)

    encT_d = nc.dram_tensor("encT", [H, T], fp16, kind="ExternalInput").ap()
    w2t_d = nc.dram_tensor("w2t", [H, H], fp16, kind="ExternalInput").ap()
    h1prm_d = nc.dram_tensor("h1prm", [H, H + BPC], fp16,
                             kind="ExternalInput").ap()
    bvec_d = nc.dram_tensor("bvec", [H], f32, kind="ExternalInput").ap()
    vcol_d = nc.dram_tensor("vcol", [KT, 128], f32, kind="ExternalInput").ap()
    onehot_d = nc.dram_tensor("onehot", [128, BPC, BPC], fp16,
                              kind="ExternalInput").ap()
    vlast_d = nc.dram_tensor("vlast", [128, HT, BPC], fp16,
                             kind="ExternalInput").ap()
    attn_d = nc.dram_tensor("attn", [BPC, L], fp16,
                            kind="ExternalOutput").ap()

    with tile.TileContext(nc) as tc:
        with ExitStack() as ctx:
            singles = ctx.enter_context(tc.tile_pool(name="singles", bufs=1))
            encp = ctx.enter_context(tc.tile_pool(name="encp", bufs=3))
            enrgp = ctx.enter_context(tc.tile_pool(name="enrgp", bufs=3))
            zp = ctx.enter_context(tc.tile_pool(name="zp", bufs=3))
            smp = ctx.enter_context(tc.tile_pool(name="smp", bufs=3))
            psP = ctx.enter_context(tc.tile_pool(name="psP", bufs=6, space="PSUM"))
            psH = ctx.enter_context(tc.tile_pool(name="psH", bufs=1, space="PSUM"))
            psS = ctx.enter_context(tc.tile_pool(name="psS", bufs=1, space="PSUM"))

            # ---- params: w2t first on sync so mm(0) can start ASAP (enc
            # group DMAs follow it on the same queue); everything else rides
            # the scalar queue and interleaves on the DMA device ----
            # PE warm-up: dependency-free dummy matmuls keep the PE busy from
            # ~0.4us until w2t lands, so the p-state ramp clock is old by the
            # time mm(0)'s matmuls are costed (full clock instead of mid/low)
            warm = singles.tile([128, GT], DT, tag="warm")
            nc.vector.memset(warm, 0.0)
            ps_w = psH.tile([1, GT], f32, tag="psh1")
            NWARM = 7
            for w in range(NWARM):
                nc.tensor.matmul(ps_w, warm[:, 0:1], warm,
                                 start=(w == 0), stop=(w == NWARM - 1))
            warmback = singles.tile([1, 1], f32, tag="warmback")
            nc.vector.tensor_copy(warmback, ps_w[:, 0:1])

            w2t_sb = singles.tile([128, KT, H], DT, tag="w2t")
            nc.sync.dma_start(out=w2t_sb,
                              in_=w2t_d.rearrange("(kt p) h -> p kt h", p=128))
            # remaining params are emitted inside the loop (after enc0) so
            # the scheduler doesn't put them ahead of enc0 on the DMA device
            b_sb = singles.tile([128, HT], f32, tag="bvec")
            h1prm_sb = singles.tile([128, KT, H + BPC], DT, tag="h1prm")
            w1t_sb = h1prm_sb[:, :, 0:H]
            hidT_sb = h1prm_sb[:, :, H:]
            vcol_sb = singles.tile([128, KT], f32, tag="vcol")
            onehot_sb = singles.tile([128, BPC, BPC], DT, tag="onehot")
            vlast_sb = singles.tile([128, HT, BPC], DT, tag="vlast")

            def emit_param_dmas_1():
                # on the sync queue AFTER enc0: the greedy ready-first tile
                # scheduler would otherwise start h1 before mm(0) on the
                # in-order PE stream, stalling mm(0) behind h1's params.
                # w1t and hidT ride one combined upload (one HWDGE gen).
                nc.scalar.dma_start(out=b_sb,
                                    in_=bvec_d.rearrange("(kt p) -> p kt",
                                                         p=128))
                nc.sync.dma_start(
                    out=h1prm_sb,
                    in_=h1prm_d.rearrange("(kt p) x -> p kt x", p=128))

            def emit_param_dmas_2():
                nc.scalar.dma_start(out=vcol_sb,
                                    in_=vcol_d.rearrange("kt p -> p kt"))
                nc.scalar.dma_start(out=onehot_sb, in_=onehot_d)
                nc.scalar.dma_start(out=vlast_sb, in_=vlast_d)

            # exp/softmax state: batch b on partition row b (one-hot columns
            # can target any PSUM row, so no 32-alignment juggling needed)
            expstore = singles.tile([BPC, L], DT, tag="expstore")
            runm0 = singles.tile([BPC, 1], f32, tag="runm0")
            nc.vector.memset(runm0, -1e30)
            runs0 = singles.tile([BPC, 1], f32, tag="runs0")
            nc.vector.memset(runs0, 0.0)
            state = {"m": runm0, "s": runs0}

            # ---- h1T[h, b] = W1T.T @ hiddenT, + bias -> SBUF f32 ----
            h1b_sb = singles.tile([128, HT, BPC], f32, tag="h1b")

            def emit_h1():
                ps_h1 = psH.tile([128, HT, BPC], f32, tag="psh1")
                for ht in range(HT):
                    for kt in range(KT):
                        nc.tensor.matmul(ps_h1[:, ht, :],
                                         w1t_sb[:, kt, ts(ht, 128)],
                                         hidT_sb[:, kt, :],
                                         start=(kt == 0), stop=(kt == KT - 1))
                for ht in range(HT):
                    nc.scalar.activation(out=h1b_sb[:, ht, :], in_=ps_h1[:, ht, :],
                                         func=ActF.Identity,
                                         bias=b_sb[:, ht:ht + 1], scale=1.0)

            # ---- batched online-softmax update after l-chunk j lands ----
            # full-rescale variant: each update rescales the already-stored
            # chunks by exp(m_old-m_new) (hidden in pipeline slack), so the
            # tail is just exp -> 1/s -> one multiply -> one DMA
            attn8 = singles.tile([BPC, L], DT, tag="attn8")

            def emit_jupdate(j, ps_sc):
                jsl = ts(j, GT)
                if j == NJ - 1:
                    # last chunk: softmax is shift-invariant, so skip the
                    # reduce_max entirely — exp against the running max
                    # (available immediately; negm kept from jupdate(2)) and
                    # let the final 1/s absorb the difference.  f32 range
                    # easily covers exp(m3-m).
                    csum = smp.tile([BPC, 1], f32, tag="csum")
                    nc.scalar.activation(out=expstore[:, jsl], in_=ps_sc,
                                         func=ActF.Exp,
                                         bias=state["negm"][:, 0:1],
                                         scale=1.0, accum_out=csum)
                    news = smp.tile([BPC, 1], f32, tag="news")
                    nc.vector.tensor_add(news, state["s"], csum)
                    state["s"] = news
                    return
                gm = smp.tile([BPC, 1], f32, tag="gm")
                nc.vector.reduce_max(out=gm, in_=ps_sc,
                                     axis=mybir.AxisListType.X)
                newm = smp.tile([BPC, 1], f32, tag="newm")
                nc.vector.tensor_max(newm, state["m"], gm)
                # rescale factor exp(m_old - m_new) for sum and stored chunks
                d = smp.tile([BPC, 1], f32, tag="d")
                nc.vector.tensor_sub(d, state["m"], newm)
                r = smp.tile([BPC, 1], f32, tag="r")
                nc.scalar.activation(out=r, in_=d, func=ActF.Exp)
                negm = smp.tile([BPC, 1], f32, tag="negm")
                nc.vector.tensor_scalar_mul(negm, newm, -1.0)
                csum = smp.tile([BPC, 1], f32, tag="csum")
                nc.scalar.activation(out=expstore[:, jsl], in_=ps_sc,
                                     func=ActF.Exp, bias=negm[:, 0:1],
                                     scale=1.0, accum_out=csum)
                if j > 0:
                    nc.vector.tensor_scalar_mul(expstore[:, 0:j * GT],
                                                expstore[:, 0:j * GT],
                                                r[:, 0:1])
                srs = smp.tile([BPC, 1], f32, tag="srs")
                nc.vector.tensor_mul(srs, state["s"], r)
                news = smp.tile([BPC, 1], f32, tag="news")
                nc.vector.tensor_add(news, srs, csum)
                state["m"], state["s"], state["negm"] = newm, news, negm

            def emit_final():
                rinv = smp.tile([BPC, 1], f32, tag="rinv")
                nc.vector.reciprocal(rinv, state["s"])
                # everything already shares the m2 reference scale, so the
                # tail is one fp16 4x multiply and one contiguous [8, L]
                # fp16 DMA (host casts the tiny result back to f32)
                nc.vector.tensor_scalar_mul(attn8, expstore, rinv[:, 0:1])
                nc.sync.dma_start(out=attn_d, in_=attn8)

            # ---- main 3-stage software pipeline, j-major over (j, b) ----
            enc_tiles = {}
            energy_tiles = {}

            def seq_bj(i):
                return i % BPC, i // BPC      # b, j

            def stage_dma(i):
                b, j = seq_bj(i)
                t = encp.tile([128, KT, GT], DT, tag="enc")
                if variant == "nodma":
                    nc.vector.memset(t[:, 0, 0:1], 0.0)
                else:
                    c0 = (b * NJ + j) * GT
                    src = encT_d[:, c0:c0 + GT].rearrange(
                        "(kt p) t -> p kt t", p=128)
                    if i == 0:
                        # group 0 in two half-chunks so mm(0) (kt-outer) can
                        # start after the first 256 k-rows instead of all 512
                        nc.sync.dma_start(out=t[:, 0:2, :], in_=src[:, 0:2, :])
                        nc.sync.dma_start(out=t[:, 2:4, :], in_=src[:, 2:4, :])
                    else:
                        nc.sync.dma_start(out=t, in_=src)
                enc_tiles[i] = t

            pre_tiles = {}

            def stage_mm_pe(i):
                st = enc_tiles.pop(i)
                pres = [psP.tile([128, GT], f32, tag="pspre",
                                 name=f"pre{i}_{ht}")
                        for ht in range(HT)]
                if i == 0:
                    # kt-outer: consume enc0's kt-chunk DMAs as they land
                    for kt in range(KT):
                        for ht in range(HT):
                            nc.tensor.matmul(pres[ht],
                                             w2t_sb[:, kt, ts(ht, 128)],
                                             st[:, kt, :],
                                             start=(kt == 0),
                                             stop=(kt == KT - 1))
                else:
                    for ht in range(HT):
                        for kt in range(KT):
                            nc.tensor.matmul(pres[ht],
                                             w2t_sb[:, kt, ts(ht, 128)],
                                             st[:, kt, :],
                                             start=(kt == 0),
                                             stop=(kt == KT - 1))
                pre_tiles[i] = pres

            def stage_mm_act(i):
                b, j = seq_bj(i)
                pres = pre_tiles.pop(i)
                en4 = enrgp.tile([128, KT, GT], DT, tag="energy")
                for ht in range(HT):
                    nc.scalar.activation(out=en4[:, ht, :], in_=pres[ht],
                                         func=ActF.Tanh,
                                         bias=h1b_sb[:, ht, b:b + 1], scale=1.0)
                energy_tiles[i] = en4

            chunk_psum = {}

            def stage_zv(i):
                b, j = seq_bj(i)
                en4 = energy_tiles.pop(i)
                if variant == "novdot":
                    return
                if b == 0:
                    chunk_psum[j] = psS.tile([BPC, GT], f32, tag="pssc",
                                             name=f"pssc{j}")
                ps_sc = chunk_psum[j]
                if i == G - 1:
                    # last group: 4 direct matmuls with v-valued one-hot
                    # stationaries — each fires right after its tanh, so no
                    # DVE chain separates the last tanh from the last vdot
                    for ht in range(HT):
                        nc.tensor.matmul(ps_sc, vlast_sb[:, ht, :],
                                         en4[:, ht, :],
                                         start=False,
                                         stop=(ht == HT - 1),
                                         skip_group_check=True)
                else:
                    # z[p, t] = sum_ht v[ht*128+p] * energy[ht][p, t]
                    # all on DVE: v[ht] is a per-partition scalar, so the
                    # muls are tensor_scalar ops in 4x mode (133ns); adds
                    # run at 2x.  (Pool/gpsimd would run at 0.42 Q7 eff.)
                    z = zp.tile([128, GT], DT, tag="z")
                    zz = zp.tile([128, KT, GT], DT, tag="zz")
                    for ht in range(HT):
                        nc.vector.tensor_scalar_mul(zz[:, ht, :],
                                                    en4[:, ht, :],
                                                    vcol_sb[:, ht:ht + 1])
                    s2 = zp.tile([128, 2, GT], DT, tag="s2")
                    nc.vector.tensor_add(s2, zz[:, 0:2, :], zz[:, 2:4, :])
                    nc.vector.tensor_add(z, s2[:, 0, :], s2[:, 1, :])
                    nc.tensor.matmul(ps_sc, onehot_sb[:, :, b], z,
                                     start=(b == 0),
                                     stop=(b == BPC - 1 and i != G - 1),
                                     skip_group_check=True)
                if b == BPC - 1 and i != G - 1:
                    emit_jupdate(j, chunk_psum.pop(j))
                if i == G - 1:
                    emit_jupdate(j, chunk_psum.pop(j))
                    emit_final()

            for it in range(G + 2):
                if it < G:
                    stage_dma(it)
                if it == 1:
                    emit_param_dmas_1()
                if 1 <= it <= G:
                    stage_mm_pe(it - 1)
                if it == 1:
                    # after mm(0) on the in-order PE queue so the big stream
                    # starts as soon as w2t+enc0 land; h1b ACT writes still
                    # precede tanh(0) on the ACT queue
                    emit_h1()
                if 1 <= it <= G:
                    stage_mm_act(it - 1)
                if it == 1:
                    emit_param_dmas_2()
                if 2 <= it:
                    stage_zv(it - 2)

    nc.compile()
    return nc


class _Runner:
    """Compile once; jit once; run many times (mirrors run_bass_via_pjrt)."""

    def __init__(self):
        import jax
        import concourse.mybir as mybir
        from concourse.bass2jax import (_bass_exec_p, install_neuronx_cc_hook,
                                        partition_id_tensor)
        from jax.sharding import Mesh, PartitionSpec
        from jax.experimental.shard_map import shard_map

        install_neuronx_cc_hook()
        nc = _build()
        self.nc = nc

        in_names, out_names, out_avals = [], [], []
        for alloc in nc.m.functions[0].allocations:
            if not isinstance(alloc, mybir.MemoryLocationSet):
                continue
            name = alloc.memorylocations[0].name
            if alloc.kind == "ExternalInput":
                in_names.append(name)
            elif alloc.kind == "ExternalOutput":
                out_names.append(name)
                out_avals.append(jax.core.ShapedArray(
                    tuple(alloc.tensor_shape), mybir.dt.np(alloc.dtype)))
        part_name = (nc.partition_id_tensor.name
                     if nc.partition_id_tensor is not None else None)
        if part_name is not None and part_name in in_names:
            in_names.remove(part_name)
        self.in_names, self.out_names, self.out_avals = in_names, out_names, out_avals
        n_params = len(in_names)
        n_outs = len(out_names)
        all_names = in_names + out_names
        if part_name is not None:
            all_names = all_names + [part_name]

        def _body(*args):
            operands = list(args)
            if part_name is not None:
                operands.append(partition_id_tensor())
            return tuple(_bass_exec_p.bind(
                *operands,
                out_avals=tuple(out_avals),
                in_names=tuple(all_names),
                out_names=tuple(out_names),
                lowering_input_output_aliases=(),
                sim_require_finite=True,
                sim_require_nnan=True,
                nc=nc,
            ))

        devices = jax.devices()[:NCORES]
        self.mesh = Mesh(np.asarray(devices), ("core",))
        in_specs = (PartitionSpec("core"),) * (n_params + n_outs)
        out_specs = (PartitionSpec("core"),) * n_outs
        self.jit = jax.jit(
            shard_map(_body, mesh=self.mesh, in_specs=in_specs,
                      out_specs=out_specs, check_rep=False),
            donate_argnums=tuple(range(n_params, n_params + n_outs)),
            keep_unused=True,
        )
        self.zero_outs = [np.zeros((NCORES * a.shape[0], *a.shape[1:]), a.dtype)
                          for a in out_avals]

    def run(self, concat_ins):
        outs = self.jit(*concat_ins, *self.zero_outs)
        return outs


_runner = None


def _get_runner():
    global _runner
    if _runner is None:
        _runner = _Runner()
    return _runner


def prepare_inputs(hidden, encoder_outputs, W, b, v):
    """Host-side shard + layout prep -> concat arrays in runner input order."""
    hidden = np.ascontiguousarray(hidden, dtype=np.float32)
    encoder_outputs = np.ascontiguousarray(encoder_outputs, dtype=np.float32)
    W = np.ascontiguousarray(W, dtype=np.float32)
    b = np.ascontiguousarray(b, dtype=np.float32)
    v = np.ascontiguousarray(v, dtype=np.float32)

    w1t = W[:, :H].T.astype(np.float16)                         # [k, h]
    w2t = np.ascontiguousarray(W[:, H:].T.astype(np.float16))   # [k, h]
    # per-core enc shard, transposed to k-major and pre-cast to fp16
    enc16 = encoder_outputs.reshape(NCORES, T, H).astype(np.float16)
    encT = np.ascontiguousarray(enc16.transpose(0, 2, 1)).reshape(NCORES * H, T)
    vcol = np.ascontiguousarray(v.reshape(KT, 128))
    onehot = np.zeros((128, BPC, BPC), np.float16)
    for bb in range(BPC):
        onehot[:, bb, bb] = 1.0
    # v-valued one-hot stationaries for the last group (b = BPC-1)
    vlast = np.zeros((128, HT, BPC), np.float16)
    vlast[:, :, BPC - 1] = v.reshape(HT, 128).T.astype(np.float16)

    h1prm = np.concatenate(
        [np.concatenate(
            [w1t, hidden[c * BPC:(c + 1) * BPC].T.astype(np.float16)], axis=1)
         for c in range(NCORES)], axis=0)
    concat = {
        "encT": encT,
        "w2t": np.tile(w2t, (NCORES, 1)),
        "h1prm": np.ascontiguousarray(h1prm),
        "bvec": np.tile(b, NCORES),
        "vcol": np.tile(vcol, (NCORES, 1)),
        "onehot": np.tile(onehot, (NCORES, 1, 1)),
        "vlast": np.tile(vlast, (NCORES, 1, 1)),
    }
    runner = _get_runner()
    return [concat[name] for name in runner.in_names]


def kernel(hidden, encoder_outputs, W, b, v):
    runner = _get_runner()
    concat_ins = prepare_inputs(hidden, encoder_outputs, W, b, v)
    outs = runner.run(concat_ins)
    (iattn,) = [i for i, n in enumerate(runner.out_names) if n == "attn"]
    attn = np.asarray(outs[iattn])          # [NCORES*BPC, L] fp16
    return attn.reshape(B, 1, L).astype(np.float32)


# revision 65
# speedup vs baseline: 1.0418x; 1.0418x over previous
#!/usr/bin/env python3
"""Bass/Trainium2 kernel for nn_Attention_12747462934680.

Reference computation (B=64, L=2048, H=512):
    x = concat([hidden broadcast over L, encoder_outputs], -1)   # [B, L, 2H]
    energy = tanh(x @ W.T + b)                                   # [B, L, H]
    scores = energy @ v                                          # [B, L]
    attn = softmax(scores, axis=1)[:, None, :]                   # [B, 1, L]

Decomposition:
    pre[b,l,h] = (enc[b,l] @ W2.T)[h] + (hidden[b] @ W1.T)[h] + bias[h]
    with W1 = W[:, :H], W2 = W[:, H:].  The hidden term is per-(b,h), computed
    once; the big matmul is enc @ W2.T.

Sharding: data-parallel over B across 8 cores (8 batches/core).

Per-core device pipeline (SPMD, no collectives).  All heavy data rides in
fp16 (~11-bit mantissa); enc is pre-transposed AND pre-cast on the host so
the kernel DMAs k-major [128, kt, t] tiles directly — no on-device
transposes at all.  Per group of 512 tokens (batch b, l-chunk j, j-outer):
  - preT[h, t] = W2T.T @ encT chunk (fp16 matmul, fp32 PSUM), 4x4 k/h tiles
  - energy = tanh(preT + h1T[:, b]) on ACT (PSUM -> SBUF, fp16), per h-tile
  - v-dot is split so the PE does 1/4 of the naive work:
      z[p, t] = sum_ht v[ht*128+p] * energy[ht][p, t]   (DVE-only tree:
      per-partition tensor_scalar muls run in 4x mode, adds in 2x)
      scores row b = onehot_b.T @ z  (single 512-row matmul; all 8 b of a
      chunk accumulate into one [8, 512] PSUM tile, one row per batch)
  - per chunk: full-rescale online softmax reads that PSUM directly:
    running max m, exp-chunk into fp16 expstore[8, L] (ACT, accum_out
    gives the chunk sum), stored chunks and s rescaled by exp(m_old-m_new)
  - the last chunk skips its reduce_max entirely (softmax is shift
    invariant, exp against the running max), and the last group's v-dots
    are direct matmuls with v-valued one-hot stationaries so no DVE chain
    separates its tanhs from the final PSUM
  - tail: exp -> s -> 1/s -> one fp16 4x multiply -> one [8, L] fp16 DMA
Latency details: a dependency-free PE warm-up chain pins the p-state ramp
clock before mm(0) is costed; w2t+enc0 lead the DMA queue so the PE stream
starts ~5us in; h1's params ride the same sync queue right after enc0 (the
tile scheduler is greedy ready-first, so h1 must not become ready first).
"""
import sys
import numpy as np

sys.path.insert(0, "/opt/trn_rl_repo")

B, L, H = 64, 2048, 512
NCORES = 8
BPC = B // NCORES          # batches per core
T = BPC * L                # tokens per core = 16384
GT = 512                   # tokens per group
G = T // GT                # 32 groups
NJ = L // GT               # 4 l-chunks per batch
KT = H // 128              # 4 k-tiles
HT = H // 128              # 4 h-tiles

_compiled = None


def _build(variant="full"):
    from contextlib import ExitStack
    from concourse import bacc, mybir, bass_isa
    import concourse.tile as tile
    from concourse.bass import ts

    f32 = mybir.dt.float32
    fp16 = mybir.dt.float16
    DT = fp16
    ActF = mybir.ActivationFunctionType

    nc = bacc.Bacc("TRN2", target_bir_lowering=False, debug=False,
                   enable_asserts=True, num_devices=NCORES)

    encT_d = nc.dram_tensor("encT", [H, T], fp16, kind="ExternalInput").ap()
    w2t_d = nc.dram_tensor("w2t", [H, H], fp16, kind="ExternalInput").ap()
    h1prm_d = nc.dram_tensor("h1prm", [H, H + BPC], fp16,
                             kind="ExternalInput").ap()
    bvec_d = nc.dram_tensor("bvec", [H], f32, kind="ExternalInput").ap()
    vcol_d = nc.dram_tensor("vcol", [KT, 128], f32, kind="ExternalInput").ap()
    onehot_d = nc.dram_tensor("onehot", [128, BPC, BPC], fp16,
                              kind="ExternalInput").ap()
    vlast_d = nc.dram_tensor("vlast", [128, HT, BPC], fp16,
                             kind="ExternalInput").ap()
    attn_d = nc.dram_tensor("attn", [BPC, L], fp16,
                            kind="ExternalOutput").ap()

    with tile.TileContext(nc) as tc:
        with ExitStack() as ctx:
            singles = ctx.enter_context(tc.tile_pool(name="singles", bufs=1))
            encp = ctx.enter_context(tc.tile_pool(name="encp", bufs=3))
            enrgp = ctx.enter_context(tc.tile_pool(name="enrgp", bufs=3))
            zp = ctx.enter_context(tc.tile_pool(name="zp", bufs=3))
            arp = ctx.enter_context(tc.tile_pool(name="arp", bufs=3))
            scp = ctx.enter_context(tc.tile_pool(name="scp", bufs=2))
            smp = ctx.enter_context(tc.tile_pool(name="smp", bufs=3))
            psP = ctx.enter_context(tc.tile_pool(name="psP", bufs=6, space="PSUM"))
            psH = ctx.enter_context(tc.tile_pool(name="psH", bufs=1, space="PSUM"))
            psS = ctx.enter_context(tc.tile_pool(name="psS", bufs=1, space="PSUM"))

            # ---- params: w2t first on sync so mm(0) can start ASAP (enc
            # group DMAs follow it on the same queue); everything else rides
            # the scalar queue and interleaves on the DMA device ----
            # PE warm-up: dependency-free dummy matmuls keep the PE busy from
            # ~0.4us until w2t lands, so the p-state ramp clock is old by the
            # time mm(0)'s matmuls are costed (full clock instead of mid/low)
            warm = singles.tile([128, GT], DT, tag="warm")
            nc.vector.memset(warm, 0.0)
            ps_w = psH.tile([1, GT], f32, tag="psh1")
            NWARM = 7
            for w in range(NWARM):
                nc.tensor.matmul(ps_w, warm[:, 0:1], warm,
                                 start=(w == 0), stop=(w == NWARM - 1))
            warmback = singles.tile([1, 1], f32, tag="warmback")
            nc.vector.tensor_copy(warmback, ps_w[:, 0:1])

            w2t_sb = singles.tile([128, KT, H], DT, tag="w2t")
            nc.sync.dma_start(out=w2t_sb,
                              in_=w2t_d.rearrange("(kt p) h -> p kt h", p=128))
            # remaining params are emitted inside the loop (after enc0) so
            # the scheduler doesn't put them ahead of enc0 on the DMA device
            b_sb = singles.tile([128, HT], f32, tag="bvec")
            h1prm_sb = singles.tile([128, KT, H + BPC], DT, tag="h1prm")
            w1t_sb = h1prm_sb[:, :, 0:H]
            hidT_sb = h1prm_sb[:, :, H:]
            vcol_sb = singles.tile([128, KT], f32, tag="vcol")
            onehot_sb = singles.tile([128, BPC, BPC], DT, tag="onehot")
            vlast_sb = singles.tile([128, HT, BPC], DT, tag="vlast")

            def emit_param_dmas_1():
                # on the sync queue AFTER enc0: the greedy ready-first tile
                # scheduler would otherwise start h1 before mm(0) on the
                # in-order PE stream, stalling mm(0) behind h1's params.
                # w1t and hidT ride one combined upload (one HWDGE gen).
                nc.scalar.dma_start(out=b_sb,
                                    in_=bvec_d.rearrange("(kt p) -> p kt",
                                                         p=128))
                nc.sync.dma_start(
                    out=h1prm_sb,
                    in_=h1prm_d.rearrange("(kt p) x -> p kt x", p=128))

            def emit_param_dmas_2():
                nc.scalar.dma_start(out=vcol_sb,
                                    in_=vcol_d.rearrange("kt p -> p kt"))
                nc.scalar.dma_start(out=onehot_sb, in_=onehot_d)
                nc.scalar.dma_start(out=vlast_sb, in_=vlast_d)

            # exp/softmax state: batch b on partition row b (one-hot columns
            # can target any PSUM row, so no 32-alignment juggling needed)
            expstore = singles.tile([BPC, L], DT, tag="expstore")
            runm0 = singles.tile([BPC, 1], f32, tag="runm0")
            nc.vector.memset(runm0, -1e30)
            runs0 = singles.tile([BPC, 1], f32, tag="runs0")
            nc.vector.memset(runs0, 0.0)
            state = {"m": runm0, "s": runs0}

            # ---- h1T[h, b] = W1T.T @ hiddenT, + bias -> SBUF f32 ----
            h1b_sb = singles.tile([128, HT, BPC], f32, tag="h1b")

            def emit_h1():
                ps_h1 = psH.tile([128, HT, BPC], f32, tag="psh1")
                for ht in range(HT):
                    for kt in range(KT):
                        nc.tensor.matmul(ps_h1[:, ht, :],
                                         w1t_sb[:, kt, ts(ht, 128)],
                                         hidT_sb[:, kt, :],
                                         start=(kt == 0), stop=(kt == KT - 1))
                for ht in range(HT):
                    nc.scalar.activation(out=h1b_sb[:, ht, :], in_=ps_h1[:, ht, :],
                                         func=ActF.Identity,
                                         bias=b_sb[:, ht:ht + 1], scale=1.0)

            # ---- batched online-softmax update after l-chunk j lands ----
            # full-rescale variant: each update rescales the already-stored
            # chunks by exp(m_old-m_new) (hidden in pipeline slack), so the
            # tail is just exp -> 1/s -> one multiply -> one DMA
            attn8 = singles.tile([BPC, L], DT, tag="attn8")

            def emit_jupdate(j, ps_sc):
                jsl = ts(j, GT)
                if j == NJ - 1:
                    # last chunk: softmax is shift-invariant, so skip the
                    # reduce_max entirely — exp against the running max
                    # (available immediately; negm kept from jupdate(2)) and
                    # let the final 1/s absorb the difference.  f32 range
                    # easily covers exp(m3-m).
                    csum = smp.tile([BPC, 1], f32, tag="csum")
                    nc.scalar.activation(out=expstore[:, jsl], in_=ps_sc,
                                         func=ActF.Exp,
                                         bias=state["negm"][:, 0:1],
                                         scale=1.0, accum_out=csum)
                    news = smp.tile([BPC, 1], f32, tag="news")
                    nc.vector.tensor_add(news, state["s"], csum)
                    state["s"] = news
                    return
                gm = smp.tile([BPC, 1], f32, tag="gm")
                nc.vector.reduce_max(out=gm, in_=ps_sc,
                                     axis=mybir.AxisListType.X)
                newm = smp.tile([BPC, 1], f32, tag="newm")
                nc.vector.tensor_max(newm, state["m"], gm)
                # rescale factor exp(m_old - m_new) for sum and stored chunks
                d = smp.tile([BPC, 1], f32, tag="d")
                nc.vector.tensor_sub(d, state["m"], newm)
                r = smp.tile([BPC, 1], f32, tag="r")
                nc.scalar.activation(out=r, in_=d, func=ActF.Exp)
                negm = smp.tile([BPC, 1], f32, tag="negm")
                nc.vector.tensor_scalar_mul(negm, newm, -1.0)
                csum = smp.tile([BPC, 1], f32, tag="csum")
                nc.scalar.activation(out=expstore[:, jsl], in_=ps_sc,
                                     func=ActF.Exp, bias=negm[:, 0:1],
                                     scale=1.0, accum_out=csum)
                if j > 0:
                    nc.vector.tensor_scalar_mul(expstore[:, 0:j * GT],
                                                expstore[:, 0:j * GT],
                                                r[:, 0:1])
                srs = smp.tile([BPC, 1], f32, tag="srs")
                nc.vector.tensor_mul(srs, state["s"], r)
                news = smp.tile([BPC, 1], f32, tag="news")
                nc.vector.tensor_add(news, srs, csum)
                state["m"], state["s"], state["negm"] = newm, news, negm

            def emit_final():
                rinv = smp.tile([BPC, 1], f32, tag="rinv")
                nc.vector.reciprocal(rinv, state["s"])
                # everything already shares the m2 reference scale, so the
                # tail is one fp16 4x multiply and one contiguous [8, L]
                # fp16 DMA (host casts the tiny result back to f32)
                nc.vector.tensor_scalar_mul(attn8, expstore, rinv[:, 0:1])
                nc.sync.dma_start(out=attn_d, in_=attn8)

            # ---- main 3-stage software pipeline, j-major over (j, b) ----
            enc_tiles = {}
            energy_tiles = {}

            def seq_bj(i):
                return i % BPC, i // BPC      # b, j

            def stage_dma(i):
                b, j = seq_bj(i)
                t = encp.tile([128, KT, GT], DT, tag="enc")
                if variant == "nodma":
                    nc.vector.memset(t[:, 0, 0:1], 0.0)
                else:
                    c0 = (b * NJ + j) * GT
                    src = encT_d[:, c0:c0 + GT].rearrange(
                        "(kt p) t -> p kt t", p=128)
                    if i == 0:
                        # group 0 in two half-chunks so mm(0) (kt-outer) can
                        # start after the first 256 k-rows instead of all 512
                        nc.sync.dma_start(out=t[:, 0:2, :], in_=src[:, 0:2, :])
                        nc.sync.dma_start(out=t[:, 2:4, :], in_=src[:, 2:4, :])
                    else:
                        nc.sync.dma_start(out=t, in_=src)
                enc_tiles[i] = t

            pre_tiles = {}

            def stage_mm_pe(i):
                st = enc_tiles.pop(i)
                pres = [psP.tile([128, GT], f32, tag="pspre",
                                 name=f"pre{i}_{ht}")
                        for ht in range(HT)]
                if i == 0:
                    # kt-outer: consume enc0's kt-chunk DMAs as they land
                    for kt in range(KT):
                        for ht in range(HT):
                            nc.tensor.matmul(pres[ht],
                                             w2t_sb[:, kt, ts(ht, 128)],
                                             st[:, kt, :],
                                             start=(kt == 0),
                                             stop=(kt == KT - 1))
                else:
                    for ht in range(HT):
                        for kt in range(KT):
                            nc.tensor.matmul(pres[ht],
                                             w2t_sb[:, kt, ts(ht, 128)],
                                             st[:, kt, :],
                                             start=(kt == 0),
                                             stop=(kt == KT - 1))
                pre_tiles[i] = pres

            def stage_mm_act(i):
                b, j = seq_bj(i)
                pres = pre_tiles.pop(i)
                en4 = enrgp.tile([128, KT, GT], DT, tag="energy")
                for ht in range(HT):
                    nc.scalar.activation(out=en4[:, ht, :], in_=pres[ht],
                                         func=ActF.Tanh,
                                         bias=h1b_sb[:, ht, b:b + 1], scale=1.0)
                energy_tiles[i] = en4

            chunk_psum = {}
            chunk_sb = {}

            def stage_zv(i):
                b, j = seq_bj(i)
                en4 = energy_tiles.pop(i)
                if variant == "novdot":
                    return
                last_chunk = (j == NJ - 1)
                if last_chunk:
                    if b == 0:
                        chunk_psum[j] = psS.tile([BPC, GT], f32, tag="pssc",
                                                 name=f"pssc{j}")
                    ps_sc = chunk_psum[j]
                if i == G - 1:
                    # last group: 4 direct matmuls with v-valued one-hot
                    # stationaries — each fires right after its tanh, so no
                    # DVE chain separates the last tanh from the last vdot
                    for ht in range(HT):
                        nc.tensor.matmul(ps_sc, vlast_sb[:, ht, :],
                                         en4[:, ht, :],
                                         start=False,
                                         stop=(ht == HT - 1),
                                         skip_group_check=True)
                    emit_jupdate(j, chunk_psum.pop(j))
                    emit_final()
                    return
                # z[p, t] = sum_ht v[ht*128+p] * energy[ht][p, t]
                # all on DVE: v[ht] is a per-partition scalar, so the
                # muls are tensor_scalar ops in 4x mode (133ns); adds
                # run at 2x.  (Pool/gpsimd tensor ops run at 0.42 Q7 eff.)
                z = zp.tile([128, GT], DT, tag="z")
                zz = zp.tile([128, KT, GT], DT, tag="zz")
                for ht in range(HT):
                    nc.vector.tensor_scalar_mul(zz[:, ht, :],
                                                en4[:, ht, :],
                                                vcol_sb[:, ht:ht + 1])
                s2 = zp.tile([128, 2, GT], DT, tag="s2")
                nc.vector.tensor_add(s2, zz[:, 0:2, :], zz[:, 2:4, :])
                nc.vector.tensor_add(z, s2[:, 0, :], s2[:, 1, :])
                if last_chunk:
                    # last chunk stays on the PE so the drain chain is short
                    nc.tensor.matmul(ps_sc, onehot_sb[:, :, b], z,
                                     start=(b == 0), stop=False,
                                     skip_group_check=True)
                    if b == BPC - 1:
                        emit_jupdate(j, chunk_psum.pop(j))
                    return
                # chunks 0..NJ-2: partition-sum on the otherwise-idle Pool
                # engine (gpsimd all-reduce); the result has the scores on
                # every partition, and a tiny SBUF->SBUF DMA moves row b
                # into the [8, GT] chunk (compute engines can't address a
                # partition offset, but DMA descriptors can)
                allred = arp.tile([128, GT], f32, tag="ar")
                nc.gpsimd.partition_all_reduce(allred, z, 128,
                                               bass_isa.ReduceOp.add)
                if b == 0:
                    chunk_sb[j] = scp.tile([BPC, GT], f32, tag="sc8",
                                           name=f"sc8_{j}")
                nc.scalar.dma_start(out=chunk_sb[j][b:b + 1, :],
                                    in_=allred[b:b + 1, :])
                if b == BPC - 1:
                    emit_jupdate(j, chunk_sb.pop(j))

            for it in range(G + 2):
                if it < G:
                    stage_dma(it)
                if it == 1:
                    emit_param_dmas_1()
                if 1 <= it <= G:
                    stage_mm_pe(it - 1)
                if it == 1:
                    # after mm(0) on the in-order PE queue so the big stream
                    # starts as soon as w2t+enc0 land; h1b ACT writes still
                    # precede tanh(0) on the ACT queue
                    emit_h1()
                if 1 <= it <= G:
                    stage_mm_act(it - 1)
                if it == 1:
                    emit_param_dmas_2()
                if 2 <= it:
                    stage_zv(it - 2)

    nc.compile()
    return nc


class _Runner:
    """Compile once; jit once; run many times (mirrors run_bass_via_pjrt)."""

    def __init__(self):
        import jax
        import concourse.mybir as mybir
        from concourse.bass2jax import (_bass_exec_p, install_neuronx_cc_hook,
                                        partition_id_tensor)
        from jax.sharding import Mesh, PartitionSpec
        from jax.experimental.shard_map import shard_map

        install_neuronx_cc_hook()
        nc = _build()
        self.nc = nc

        in_names, out_names, out_avals = [], [], []
        for alloc in nc.m.functions[0].allocations:
            if not isinstance(alloc, mybir.MemoryLocationSet):
                continue
            name = alloc.memorylocations[0].name
            if alloc.kind == "ExternalInput":
                in_names.append(name)
            elif alloc.kind == "ExternalOutput":
                out_names.append(name)
                out_avals.append(jax.core.ShapedArray(
                    tuple(alloc.tensor_shape), mybir.dt.np(alloc.dtype)))
        part_name = (nc.partition_id_tensor.name
                     if nc.partition_id_tensor is not None else None)
        if part_name is not None and part_name in in_names:
            in_names.remove(part_name)
        self.in_names, self.out_names, self.out_avals = in_names, out_names, out_avals
        n_params = len(in_names)
        n_outs = len(out_names)
        all_names = in_names + out_names
        if part_name is not None:
            all_names = all_names + [part_name]

        def _body(*args):
            operands = list(args)
            if part_name is not None:
                operands.append(partition_id_tensor())
            return tuple(_bass_exec_p.bind(
                *operands,
                out_avals=tuple(out_avals),
                in_names=tuple(all_names),
                out_names=tuple(out_names),
                lowering_input_output_aliases=(),
                sim_require_finite=True,
                sim_require_nnan=True,
                nc=nc,
            ))

        devices = jax.devices()[:NCORES]
        self.mesh = Mesh(np.asarray(devices), ("core",))
        in_specs = (PartitionSpec("core"),) * (n_params + n_outs)
        out_specs = (PartitionSpec("core"),) * n_outs
        self.jit = jax.jit(
            shard_map(_body, mesh=self.mesh, in_specs=in_specs,
                      out_specs=out_specs, check_rep=False),
            donate_argnums=tuple(range(n_params, n_params + n_outs)),
            keep_unused=True,
        )
        self.zero_outs = [np.zeros((NCORES * a.shape[0], *a.shape[1:]), a.dtype)
                          for a in out_avals]

    def run(self, concat_ins):
        outs = self.jit(*concat_ins, *self.zero_outs)
        return outs


_runner = None


def _get_runner():
    global _runner
    if _runner is None:
        _runner = _Runner()
    return _runner


def prepare_inputs(hidden, encoder_outputs, W, b, v):
    """Host-side shard + layout prep -> concat arrays in runner input order."""
    hidden = np.ascontiguousarray(hidden, dtype=np.float32)
    encoder_outputs = np.ascontiguousarray(encoder_outputs, dtype=np.float32)
    W = np.ascontiguousarray(W, dtype=np.float32)
    b = np.ascontiguousarray(b, dtype=np.float32)
    v = np.ascontiguousarray(v, dtype=np.float32)

    w1t = W[:, :H].T.astype(np.float16)                         # [k, h]
    w2t = np.ascontiguousarray(W[:, H:].T.astype(np.float16))   # [k, h]
    # per-core enc shard, transposed to k-major and pre-cast to fp16
    enc16 = encoder_outputs.reshape(NCORES, T, H).astype(np.float16)
    encT = np.ascontiguousarray(enc16.transpose(0, 2, 1)).reshape(NCORES * H, T)
    vcol = np.ascontiguousarray(v.reshape(KT, 128))
    onehot = np.zeros((128, BPC, BPC), np.float16)
    for bb in range(BPC):
        onehot[:, bb, bb] = 1.0
    # v-valued one-hot stationaries for the last group (b = BPC-1)
    vlast = np.zeros((128, HT, BPC), np.float16)
    vlast[:, :, BPC - 1] = v.reshape(HT, 128).T.astype(np.float16)

    h1prm = np.concatenate(
        [np.concatenate(
            [w1t, hidden[c * BPC:(c + 1) * BPC].T.astype(np.float16)], axis=1)
         for c in range(NCORES)], axis=0)
    concat = {
        "encT": encT,
        "w2t": np.tile(w2t, (NCORES, 1)),
        "h1prm": np.ascontiguousarray(h1prm),
        "bvec": np.tile(b, NCORES),
        "vcol": np.tile(vcol, (NCORES, 1)),
        "onehot": np.tile(onehot, (NCORES, 1, 1)),
        "vlast": np.tile(vlast, (NCORES, 1, 1)),
    }
    runner = _get_runner()
    return [concat[name] for name in runner.in_names]


def kernel(hidden, encoder_outputs, W, b, v):
    runner = _get_runner()
    concat_ins = prepare_inputs(hidden, encoder_outputs, W, b, v)
    outs = runner.run(concat_ins)
    (iattn,) = [i for i, n in enumerate(runner.out_names) if n == "attn"]
    attn = np.asarray(outs[iattn])          # [NCORES*BPC, L] fp16
    return attn.reshape(B, 1, L).astype(np.float32)
